# revision 1
# baseline (speedup 1.0000x reference)
"""EventDenoisingMamba Trainium2 kernel (Bass/Tile), batch-parallel over 8 cores.

Layout: d-major (feature dim on partitions, time on the free axis).
Per layer, per time-chunk of T:
  - in_proj with the depthwise causal conv folded in as a K=4*DM matmul
    (W_eff[(k,m),dout] = conv_w[dout,k]*in_w[dout,m]) on PE, silu on ACT
  - x_proj / dt_proj on PE, softplus on ACT
  - dA_s = exp(A_s * dt) on ACT (scalar scale; A is d-independent by construction)
  - b_s = dtx * B_rep_s on GP/DVE; B_rep via DMA partition-broadcast
  - selective scan: hardware tensor_tensor_scan per (s, db) plane on DVE
  - w_s = h_s * C_rep_s; y = sum_s w_s + xc*D via identity-matmul PSUM
    accumulation on PE
  - gate with silu(z), out_proj on PE
"""
from contextlib import ExitStack

import numpy as np

import concourse.bass as bass
import concourse.bacc as bacc
import concourse.tile as tile
import concourse.mybir as mybir

FP32 = mybir.dt.float32
BF16 = mybir.dt.bfloat16
MULT = mybir.AluOpType.mult
ADD = mybir.AluOpType.add
AF = mybir.ActivationFunctionType

DM, DI, DS, DC, DTR = 256, 512, 16, 4, 16
NDB = DI // 128          # 4 d-blocks
NMH = DM // 128          # 2 m-halves


def build(nc, L, T, NL, a_scalars, dve_ymul_s=12, dve_bgen_s=4, plane_bufs=3, rep_bufs=2, debug=False, d_is_one=True):
    """Emit the kernel IR. a_scalars: list of DS python floats (A[s], d-indep).
    Returns nothing; declares DRAM tensors by name."""
    NC = L // T  # chunks
    inp = {}

    def din(name, shape, dt):
        inp[name] = nc.dram_tensor(name, shape, dt, kind="ExternalInput").ap()
        return inp[name]

    featT = din("featT", [11, L], BF16)
    emb_w = din("emb_w", [11, DM], BF16)             # lhsT [k, m]
    emb_b = din("emb_b", [128, NMH], FP32)           # bias cols
    w_eff = din("w_eff", [128, NL, 2 * DC, DI], BF16)  # lhsT K-blocks
    inw_z = din("inw_z", [128, NL, NMH, DI], BF16)
    conv_b = din("conv_b", [128, NL, NDB], FP32)
    nconv_b = din("nconv_b", [128, NL, NDB], FP32)
    xp_w = din("xp_w", [128, NL, NDB, 48], BF16)
    dtp_w = din("dtp_w", [DTR, NL, DI], BF16)
    dtp_b = din("dtp_b", [128, NL, NDB], FP32)
    d_col = din("d_col", [128, NL, NDB], FP32)
    outw = din("outw", [128, NL, NDB, DM], BF16)
    head_w = din("head_w", [128, NMH, 1], BF16)
    head_b = din("head_b", [1, 1], FP32)
    nhead_b = din("nhead_b", [1, 1], FP32)
    ident = din("ident", [128, 128], BF16)
    out = nc.dram_tensor("out", [1, L], FP32, kind="ExternalOutput").ap()
    dbgx = (nc.dram_tensor("dbgx", [128, NMH, L], BF16, kind="ExternalOutput").ap()
            if debug else None)

    with ExitStack() as ctx:
        P = lambda name, bufs, **kw: ctx.enter_context(
            tc.tile_pool(name=name, bufs=bufs, **kw))
        tc = ctx.enter_context(tile.TileContext(nc))
        wp = P("wp", 1)
        xpool = P("x", 1)
        work = P("work", 2)
        plane = P("plane", plane_bufs)
        rep = P("rep", rep_bufs)
        drp = P("drp", 4, space="DRAM")
        psum = P("psum", 2, space="PSUM")
        psum_y = P("psum_y", 4, space="PSUM")
        psum_s = P("psum_s", 1, space="PSUM")

        # ---- load weights to SBUF ----
        def wtile(ap, nm):
            t = wp.tile(list(ap.shape), ap.dtype, name=nm, tag=nm)
            nc.sync.dma_start(out=t[:], in_=ap)
            return t

        s_featT = wtile(featT, "s_featT")
        s_embw = wtile(emb_w, "s_embw")
        s_embb = wtile(emb_b, "s_embb")
        s_weff = wtile(w_eff, "s_weff")
        s_inwz = wtile(inw_z, "s_inwz")
        s_convb = wtile(conv_b, "s_convb")
        s_nconvb = wtile(nconv_b, "s_nconvb")
        s_xpw = wtile(xp_w, "s_xpw")
        s_dtpw = wtile(dtp_w, "s_dtpw")
        s_dtpb = wtile(dtp_b, "s_dtpb")
        s_dcol = wtile(d_col, "s_dcol")
        s_outw = wtile(outw, "s_outw")
        s_headw = wtile(head_w, "s_headw")
        s_headb = wtile(head_b, "s_headb")
        s_nheadb = wtile(nhead_b, "s_nheadb")
        s_ident = wtile(ident, "s_ident")

        # ---- x ping-pong chunk tiles [128, NMH, T+3] ----
        xbuf = [[xpool.tile([128, NMH, T + 3], BF16, tag=f"x{p}_{c}", name=f"x{p}_{c}")
                 for c in range(NC)] for p in range(2)]
        for p in range(2):
            nc.vector.memset(xbuf[p][0][:, :, 0:3], 0.0)

        carry = wp.tile([128, DS, NDB, 1], BF16)

        def write_x(dst_p, c, mo, psrc):
            """psrc: PSUM [128, T] -> x tile c cols 3.. + tail into c+1."""
            nc.scalar.activation(out=xbuf[dst_p][c][:, mo, 3:3 + T], in_=psrc,
                                 func=AF.Copy)
            if c + 1 < NC:
                nc.scalar.activation(out=xbuf[dst_p][c + 1][:, mo, 0:3],
                                     in_=psrc[:, T - 3:T], func=AF.Copy)

        # ---- embedding ----
        for c in range(NC):
            for mo in range(NMH):
                pe = psum.tile([128, T], FP32, tag="mm")
                nc.tensor.matmul(pe[:], s_embw[:, mo * 128:(mo + 1) * 128],
                                 s_featT[:, c * T:(c + 1) * T],
                                 start=True, stop=True)
                # add bias via activation bias column
                pb = work.tile([128, T], FP32, tag="embtmp")
                nc.scalar.activation(out=pb[:], in_=pe[:], func=AF.Identity,
                                     bias=s_embb[:, mo:mo + 1], scale=1.0)
                nc.scalar.activation(out=xbuf[0][c][:, mo, 3:3 + T], in_=pb[:],
                                     func=AF.Copy)
                if c + 1 < NC:
                    nc.scalar.activation(out=xbuf[0][c + 1][:, mo, 0:3],
                                         in_=pb[:, T - 3:T], func=AF.Copy)

        # ---- layers (software-pipelined: front(i+1) emitted before back(i)
        # so PE's in-order queue has next-chunk projection matmuls ready while
        # this chunk's scan-paced identity-matmuls drain) ----

        def front(l, c):
            src = l % 2
            xc_t = work.tile([128, NDB, T], BF16, tag="xc", name="xc_t")
            zs_t = work.tile([128, NDB, T], BF16, tag="zs", name="zs_t")
            dt_t = work.tile([128, NDB, T], BF16, tag="dt", name="dt_t")
            dtx_t = work.tile([128, NDB, T], BF16, tag="dtx", name="dtx_t")
            xdbl = work.tile([48, T], BF16, tag="xdbl", name="xdbl")
            xt = xbuf[src][c]
            pre_xc = work.tile([128, NDB, T], BF16, tag="pre", name="pre_xc")
            e_xc = work.tile([128, NDB, T], BF16, tag="e", name="e_xc")
            pre_z = work.tile([128, NDB, T], BF16, tag="pre", name="pre_z")
            e_z = work.tile([128, NDB, T], BF16, tag="e", name="e_z")
            for m in range(NDB):
                pmm = psum.tile([128, T], FP32, tag="mm", name="pmm")
                for kb in range(2 * DC):
                    k, mh = kb >> 1, kb & 1
                    nc.tensor.matmul(
                        pmm[:], s_weff[:, l, kb, m * 128:(m + 1) * 128],
                        xt[:, mh, k:k + T],
                        start=(kb == 0), stop=(kb == 2 * DC - 1))
                nc.scalar.activation(out=pre_xc[:, m, :], in_=pmm[:],
                                     func=AF.Identity,
                                     bias=s_convb[:, l, m:m + 1], scale=1.0)
                nc.scalar.activation(out=e_xc[:, m, :], in_=pmm[:],
                                     func=AF.Exp,
                                     bias=s_nconvb[:, l, m:m + 1], scale=-1.0)
            nc.scalar.activation(out=e_xc[:], in_=e_xc[:], func=AF.Ln,
                                 bias=1.0, scale=1.0)
            nc.scalar.activation(out=e_xc[:], in_=e_xc[:], func=AF.Exp,
                                 bias=0.0, scale=-1.0)
            nc.gpsimd.tensor_tensor(out=xc_t[:], in0=pre_xc[:],
                                    in1=e_xc[:], op=MULT)
            for m in range(NDB):
                pmm = psum.tile([128, T], FP32, tag="mm", name="pmm")
                for mh in range(NMH):
                    nc.tensor.matmul(
                        pmm[:], s_inwz[:, l, mh, m * 128:(m + 1) * 128],
                        xt[:, mh, 3:3 + T],
                        start=(mh == 0), stop=(mh == NMH - 1))
                nc.scalar.activation(out=pre_z[:, m, :], in_=pmm[:],
                                     func=AF.Copy)
                nc.scalar.activation(out=e_z[:, m, :], in_=pmm[:],
                                     func=AF.Exp, bias=0.0, scale=-1.0)
            nc.scalar.activation(out=e_z[:], in_=e_z[:], func=AF.Ln,
                                 bias=1.0, scale=1.0)
            nc.scalar.activation(out=e_z[:], in_=e_z[:], func=AF.Exp,
                                 bias=0.0, scale=-1.0)
            nc.gpsimd.tensor_tensor(out=zs_t[:], in0=pre_z[:],
                                    in1=e_z[:], op=MULT)
            pxp = psum_s.tile([48, T], FP32, tag="xp", name="pxp")
            for db in range(NDB):
                nc.tensor.matmul(pxp[:], s_xpw[:, l, db, :], xc_t[:, db, :],
                                 start=(db == 0), stop=(db == NDB - 1))
            nc.scalar.activation(out=xdbl[:], in_=pxp[:], func=AF.Copy)
            for m in range(NDB):
                pmm = psum.tile([128, T], FP32, tag="mm", name="pmm")
                nc.tensor.matmul(pmm[:], s_dtpw[:, l, m * 128:(m + 1) * 128],
                                 xdbl[0:DTR, :], start=True, stop=True)
                nc.scalar.activation(out=dt_t[:, m, :], in_=pmm[:],
                                     func=AF.Exp,
                                     bias=s_dtpb[:, l, m:m + 1], scale=1.0)
            nc.scalar.activation(out=dt_t[:], in_=dt_t[:], func=AF.Ln,
                                 bias=1.0, scale=1.0)
            nc.gpsimd.tensor_tensor(out=dtx_t[:], in0=dt_t[:],
                                    in1=xc_t[:], op=MULT)
            if d_is_one:
                xcd_t = xc_t
            else:
                xcd_t = work.tile([128, NDB, T], BF16, tag="xcd", name="xcd_t")
                for m in range(NDB):
                    nc.scalar.activation(out=xcd_t[:, m, :], in_=xc_t[:, m, :],
                                         func=AF.Identity, bias=0.0,
                                         scale=s_dcol[:, l, m:m + 1])
            xdbl_d = drp.tile([2 * DS, T], BF16, name="xdbl_d")
            nc.sync.dma_start(out=xdbl_d[:], in_=xdbl[DTR:DTR + 2 * DS, :])
            brep, crep = [], []
            for s in range(DS):
                bt = rep.tile([128, T], BF16, tag=f"brep{s % 2}", name=f"brep{s}")
                nc.sync.dma_start(
                    out=bt[:], in_=xdbl_d[s:s + 1, :].to_broadcast([128, T]))
                brep.append(bt)
                ct = rep.tile([128, T], BF16, tag=f"crep{s % 2}", name=f"crep{s}")
                nc.sync.dma_start(
                    out=ct[:],
                    in_=xdbl_d[DS + s:DS + s + 1, :].to_broadcast([128, T]))
                crep.append(ct)
            return dict(xc=xc_t, zs=zs_t, dt=dt_t, dtx=dtx_t, xcd=xcd_t,
                        brep=brep, crep=crep)

        def back(l, c, st):
            dst = (l + 1) % 2
            gated = work.tile([128, NDB, T], BF16, tag="gated", name="gated")
            py = [psum_y.tile([128, T], FP32, tag="y", name=f"py{db}")
                  for db in range(NDB)]
            for s in range(DS):
                da_t = plane.tile([128, NDB, T], BF16, tag="dA", name="da_t")
                b_t = plane.tile([128, NDB, T], BF16, tag="b", name="b_t")
                h_t = plane.tile([128, NDB, T], BF16, tag="h", name="h_t")
                w_t = plane.tile([128, NDB, T], BF16, tag="dA", name="w_t")
                nc.scalar.activation(out=da_t[:], in_=st["dt"][:], func=AF.Exp,
                                     scale=float(a_scalars[l][s]))
                beng = nc.vector if s < dve_bgen_s else nc.gpsimd
                beng.tensor_tensor(
                    out=b_t[:], in0=st["dtx"][:],
                    in1=st["brep"][s][:, None, :].broadcast_to([128, NDB, T]),
                    op=MULT)
                for db in range(NDB):
                    ini = 0.0 if c == 0 else carry[:, s, db, :]
                    nc.vector.tensor_tensor_scan(
                        h_t[:, db, :], da_t[:, db, :], b_t[:, db, :],
                        ini, MULT, ADD)
                nc.scalar.activation(out=carry[:, s, :, :],
                                     in_=h_t[:, :, T - 1:T], func=AF.Copy)
                weng = nc.gpsimd if s < (DS - dve_ymul_s) else nc.vector
                weng.tensor_tensor(
                    out=w_t[:], in0=h_t[:],
                    in1=st["crep"][s][:, None, :].broadcast_to([128, NDB, T]),
                    op=MULT)
                for db in range(NDB):
                    nc.tensor.matmul(py[db][:], s_ident[:], w_t[:, db, :],
                                     start=(s == 0), stop=False)
            for db in range(NDB):
                nc.tensor.matmul(py[db][:], s_ident[:], st["xcd"][:, db, :],
                                 start=False, stop=True)
                nc.vector.tensor_tensor(out=gated[:, db, :], in0=py[db][:],
                                        in1=st["zs"][:, db, :], op=MULT)
            for mo in range(NMH):
                pmm = psum.tile([128, T], FP32, tag="mm", name="pmm")
                for db in range(NDB):
                    nc.tensor.matmul(
                        pmm[:], s_outw[:, l, db, mo * 128:(mo + 1) * 128],
                        gated[:, db, :],
                        start=(db == 0), stop=(db == NDB - 1))
                write_x(dst, c, mo, pmm[:])

        seq = [(l, c) for l in range(NL) for c in range(NC)]
        pending = front(*seq[0])
        for i in range(len(seq)):
            nxt = front(*seq[i + 1]) if i + 1 < len(seq) else None
            back(*seq[i], pending)
            pending = nxt

        # ---- head ----
        fin = NL % 2
        if debug:
            for c in range(NC):
                nc.sync.dma_start(out=dbgx[:, :, c * T:(c + 1) * T],
                                  in_=xbuf[fin][c][:, :, 3:3 + T])
        for c in range(NC):
            ph = psum_s.tile([1, T], FP32, tag="head")
            for mo in range(NMH):
                nc.tensor.matmul(ph[:], s_headw[:, mo, :],
                                 xbuf[fin][c][:, mo, 3:3 + T],
                                 start=(mo == 0), stop=(mo == NMH - 1))
            ot = work.tile([1, T], FP32, tag="out")
            nc.scalar.activation(out=ot[:], in_=ph[:], func=AF.Exp,
                                 bias=s_nheadb[0:1, 0:1], scale=-1.0)
            nc.scalar.activation(out=ot[:], in_=ot[:], func=AF.Ln,
                                 bias=1.0, scale=1.0)
            nc.scalar.activation(out=ot[:], in_=ot[:], func=AF.Exp,
                                 bias=0.0, scale=-1.0)
            nc.sync.dma_start(out=out[0:1, c * T:(c + 1) * T], in_=ot[0:1, :])


def pack_inputs(f, core, L, NL):
    """Host-side packing of full inputs -> per-core DRAM input dict."""
    bf = lambda a: np.ascontiguousarray(a, np.float32).astype(
        np.dtype("bfloat16") if False else np.float32)
    # ml_dtypes bfloat16 via numpy
    import ml_dtypes
    tobf = lambda a: np.asarray(a, np.float32).astype(ml_dtypes.bfloat16)
    f32 = lambda a: np.ascontiguousarray(np.asarray(a, np.float32))

    d = {}
    d["featT"] = tobf(f["features"][core, :L].T)                    # [11, L]
    d["emb_w"] = tobf(f["emb_w"].T)                                 # [11, 256]
    ebc = np.zeros((128, NMH), np.float32)
    for mo in range(NMH):
        ebc[:, mo] = f["emb_b"][mo * 128:(mo + 1) * 128]
    d["emb_b"] = ebc
    weff = np.zeros((128, NL, 2 * DC, DI), np.float32)
    inwz = np.zeros((128, NL, NMH, DI), np.float32)
    convb = np.zeros((128, NL, NDB), np.float32)
    xpw = np.zeros((128, NL, NDB, 48), np.float32)
    dtpw = np.zeros((DTR, NL, DI), np.float32)
    dtpb = np.zeros((128, NL, NDB), np.float32)
    dcol = np.zeros((128, NL, NDB), np.float32)
    outw = np.zeros((128, NL, NDB, DM), np.float32)
    for l in range(NL):
        in_w = np.asarray(f["in_w"][l], np.float32)     # [1024, 256]
        conv_w = np.asarray(f["conv_w"][l], np.float32)  # [512, 4]
        for kb in range(2 * DC):
            k, mh = kb >> 1, kb & 1
            # lhsT[p, dout] = conv_w[dout, k] * in_w[dout, mh*128+p]
            weff[:, l, kb, :] = (conv_w[:, k] * in_w[:DI, mh * 128:(mh + 1) * 128].T)
        for mh in range(NMH):
            inwz[:, l, mh, :] = in_w[DI:, mh * 128:(mh + 1) * 128].T
        for db in range(NDB):
            convb[:, l, db] = f["conv_b"][l][db * 128:(db + 1) * 128]
            dtpb[:, l, db] = f["dtp_b"][l][db * 128:(db + 1) * 128]
            dcol[:, l, db] = f["D"][l][db * 128:(db + 1) * 128]
            xpw[:, l, db, :] = np.asarray(f["xp_w"][l], np.float32)[:, db * 128:(db + 1) * 128].T
        dtpw[:, l, :] = np.asarray(f["dtp_w"][l], np.float32).T
        outw_l = np.asarray(f["out_w"][l], np.float32)  # [256, 512]
        for db in range(NDB):
            outw[:, l, db, :] = outw_l[:, db * 128:(db + 1) * 128].T
    d["w_eff"] = tobf(weff)
    d["inw_z"] = tobf(inwz)
    d["conv_b"] = convb
    d["nconv_b"] = -convb
    d["xp_w"] = tobf(xpw)
    d["dtp_w"] = tobf(dtpw)
    d["dtp_b"] = dtpb
    d["d_col"] = dcol
    d["outw"] = tobf(outw)
    hw = np.zeros((128, NMH, 1), np.float32)
    for mo in range(NMH):
        hw[:, mo, 0] = np.asarray(f["head_w"], np.float32)[0, mo * 128:(mo + 1) * 128]
    d["head_w"] = tobf(hw)
    d["head_b"] = f32(f["head_b"]).reshape(1, 1)
    d["nhead_b"] = -f32(f["head_b"]).reshape(1, 1)
    d["ident"] = tobf(np.eye(128, dtype=np.float32))
    return d


def a_scalars_from(f, NL):
    A = -np.exp(np.asarray(f["A_log"], np.float32))  # [NL, DI, DS]
    # d-independent check (true for S4D-real init)
    sc = []
    for l in range(NL):
        assert np.allclose(A[l], A[l][0:1, :], rtol=0, atol=0), "A d-dep!"
    # identical across layers too for this model, but keep per-layer list
    return [[float(A[l][0, s]) for s in range(DS)] for l in range(NL)]


# ----------------------------------------------------------------------------
# Public entry: kernel(**inputs) -> [8, 4096, 1] float32
# ----------------------------------------------------------------------------
_CACHE = {}

# Force a single ACT table containing Exp+Ln+Copy+Identity so bacc never
# alternates table loads between Exp and Ln instructions.
import concourse.bacc as _bacc_mod
_orig_tables = _bacc_mod.get_activation_tables


def _single_table(arch):
    t = _orig_tables(arch)
    shared = {AF.Exp, AF.Ln, AF.Copy, AF.Identity, AF.MemsetZero}
    out = {}
    for k, v in t.items():
        if k == "natural_log_exp_and_others":
            out[k] = v
        else:
            out[k] = {f for f in v if f not in shared}
    return out


_bacc_mod.get_activation_tables = _single_table

L_FULL, T_FULL, NL_FULL, N_CORES = 4096, 512, 4, 8


def _get_compiled(a_sc, d_is_one):
    key = "k"
    if key not in _CACHE:
        nc = bacc.Bacc("TRN2", target_bir_lowering=False, debug=False,
                       num_devices=N_CORES)
        build(nc, L_FULL, T_FULL, NL_FULL, a_sc, d_is_one=d_is_one)
        nc.compile()
        _CACHE[key] = nc
    return _CACHE[key]


def kernel(**inputs):
    from concourse import bass_utils
    f = {k: np.asarray(v) for k, v in inputs.items()}
    A = -np.exp(np.asarray(f["A_log"], np.float32))
    assert np.allclose(A, A[:, 0:1, :]), "A must be d-independent"
    a_sc = [[float(A[l][0, s]) for s in range(DS)] for l in range(NL_FULL)]
    d_is_one = bool(np.all(np.asarray(f["D"], np.float32) == 1.0))
    assert d_is_one, "fast path assumes D == 1"
    nc = _get_compiled(a_sc, d_is_one)
    in_maps = [pack_inputs(f, core, L_FULL, NL_FULL) for core in range(N_CORES)]
    res = bass_utils.run_bass_kernel_spmd(nc, in_maps,
                                          core_ids=list(range(N_CORES)))
    out = np.stack([res.results[c]["out"].reshape(L_FULL, 1)
                    for c in range(N_CORES)])
    return out.astype(np.float32)

